# revision 37
# baseline (speedup 1.0000x reference)
"""Trainium2 Bass kernel for nn_AttnBlock: GroupNorm -> single-head spatial
self-attention (QKV 1x1 convs, softmax over 1024 positions, AV) -> proj 1x1
conv -> residual.

Sharding: data-parallel over batch. B=16 -> 2 batches per NeuronCore x 8
cores; identical NEFF per core; host gathers.

v3: mixed fp8(e4m3, DoubleRow) / bf16 pipeline.

Score path folds Wq^T Wk into one host-precomputed matrix M (scores =
(Wq h)^T (Wk h) = h^T (M h)), which both removes the separate Q and K
projections (-16 matmuls/batch) and removes two quantization sites (the
q and k stores). The t = M h projection, the scores h^T t matmul, V, and
the output projection all run as fp8 DoubleRow (2 contraction rows per PE
cell, 2 fp8 cols/cycle: 2x the bf16 rate). The value path after softmax --
exp output a, v, the a*v matmul and the softmax-denominator matmul -- runs
in bf16 at the normal rate: fp8 attention weights were the next-largest
error contributor, and this split lands end-to-end error ~1.3e-2 vs the
2e-2 gate (all-fp8 measured 2.22e-2).

DoubleRow operand layout is [128, 2, F]: two 128-row contraction blocks
stacked contiguously in the free dim, so every evacuation writes plain
contiguous [128,1024] slices:
    h2t[cp]  [128, 2*N]   fp8   GN output, channel tiles (2cp, 2cp+1)
    t2[cp][nch] [128,1024] fp8  t = M h, c-tiles (2cp,2cp+1) x query chunk
    vt2[np]  [128, 1024]  bf16  v^T, key-tiles (2np, 2np+1) x 512 channels
    at2[kp]  [128, 2*N]   bf16  exp(scores^T), key-tiles (2kp, 2kp+1)
    h2q[cp]  [128, 2*N]   fp8   attention output, channel tiles (2cp,2cp+1)

fp8 scales (TRN e4m3 max=240; host clips to 224): M x32 (entries ~C^-0.5
would land subnormal), so t carries x32, folded out in the exp scale.
wv, wp x16; v and h2 carry x16; the proj psum carries x256, descaled in
the residual-add evacuation. exp gets bias -1.5 (softmax shift-invariant).

Softmax skips max-subtraction (logits ~N(0,1), |logit|<6). The denominator
is an all-ones matmul that lands the key-sum broadcast across all 128
partitions, exactly what the normalizing multiply wants. GroupNorm group
stats (16 channels/group on partitions) use one tiny fp32 matmul against a
block-diagonal averaging matrix; all four channel tiles' stats ride in
single [128,4]-wide DVE ops, with rstd via Newton rsqrt on DVE (bit-trick
seed + 2 iterations) so ACT runs only {Square, Identity, Exp} -- one
exp_and_others table load per program (v1 paid 2 switches x 2.7us per
batch ping-ponging Sqrt/Exp sets).

Engine budget per batch (errata-adjusted): PE ~34us (82k cycles), DVE
~19us, ACT ~21us; PE is the roofline.
"""

import os
import sys

import numpy as np

for _p in ("/opt/trn_rl_repo", "/root/.axon_site/_ro/trn_rl_repo"):
    if os.path.isdir(_p) and _p not in sys.path:
        sys.path.insert(0, _p)

import concourse.bacc as bacc
import concourse.tile as tile
import concourse.mybir as mybir
from concourse.alu_op_type import AluOpType
from concourse.bass_utils import run_bass_kernel_spmd

B, C, H, W = 16, 512, 32, 32
N = H * W                  # 1024 spatial positions
GROUPS = 32
GS = C // GROUPS           # 16 channels per group
NCORES = 8
BPC = B // NCORES          # batches per core
CT = C // 128              # channel 128-tiles
CP = CT // 2               # channel 256-pair groups (DoubleRow)
NT = N // 128              # position 128-tiles
KP = NT // 2               # key 256-pair groups
NCH = N // 512             # 512-wide query chunks
EPS = 1e-5
ATTN_SCALE = float(C) ** -0.5
SW = 16.0                  # fp8 scale for wv/wp
SM = 32.0                  # fp8 scale for M = Wq^T Wk
EXP_BIAS = -1.5            # softmax shift: keeps exp() outputs small

F32 = mybir.dt.float32
BF16 = mybir.dt.bfloat16
FP8 = mybir.dt.float8e4
I32 = mybir.dt.int32
DR = mybir.MatmulPerfMode.DoubleRow
Act = mybir.ActivationFunctionType

LAST_RESULTS = None        # BassKernelResults of the most recent run (for test.py)

_PROGRAM_CACHE = {}


def _build_program(flags=(), loop_reps=None, bench_internal=False):
    """Build the per-core Bass program.

    loop_reps: if set, wrap the whole per-core body in a hardware For_i loop
    executing it that many times (benchmarking only -- output is identical
    every iteration since the program re-reads xs).

    bench_internal: benchmarking only -- declare all big tensors as Internal
    DRAM (zero-filled by a one-time prologue) so timed calls move no host
    data; the program computes on zeros but executes identical instructions."""
    nc = bacc.Bacc(
        "TRN2",
        target_bir_lowering=False,
        debug=False,
        enable_asserts=False,
        num_devices=NCORES,
    )

    kind = "Internal" if bench_internal else "ExternalInput"

    def din(name, shape, dt=F32):
        return nc.dram_tensor(name, shape, dt, kind=kind).ap()

    xs = din("xs", [BPC, CT, 128, N])
    wm = din("wm2", [CP, 128, 2 * C], FP8)
    wv = din("wv2", [CP, 128, 2 * C], FP8)
    wp = din("wp2", [CP, 128, 2 * C], FP8)
    gnw = din("gnw", [128, CT])
    gnb = din("gnb", [128, CT])
    ones_d = din("ones1", [128, 128], BF16)
    gmat_d = din("gmat", [128, 128])

    out_kind = "Internal" if bench_internal else "ExternalOutput"
    out_d = nc.dram_tensor("out", [BPC, CT, 128, N], F32, kind=out_kind).ap()
    sink_d = (nc.dram_tensor("sink", [1, 4], F32, kind="ExternalOutput").ap()
              if bench_internal else None)

    with tile.TileContext(nc) as tc:
        if bench_internal:
            with tc.tile_pool(name="zfill", bufs=1) as zp:
                zt = zp.tile([128, N], F32, tag="z", name="zt")
                nc.vector.memset(zt, 0.01)
                for b_ in range(BPC):
                    for t_ in range(CT):
                        nc.sync.dma_start(out=xs[b_, t_], in_=zt)
                for w_ in (wm, wv, wp):
                    for cp_ in range(CP):
                        nc.sync.dma_start(out=w_[cp_],
                                          in_=zt.bitcast(FP8)[:, 0:2 * C])
                nc.sync.dma_start(out=gnw, in_=zt[:, 0:CT])
                nc.sync.dma_start(out=gnb, in_=zt[:, 0:CT])
                nc.sync.dma_start(out=ones_d, in_=zt.bitcast(BF16)[:, 0:128])
                nc.sync.dma_start(out=gmat_d, in_=zt[:, 0:128])
                nc.sync.dma_start(out=sink_d, in_=zt[0:1, 0:4])
        _emit(tc, xs, wm, wv, wp, gnw, gnb, ones_d, gmat_d, out_d,
              loop_reps=loop_reps)
    nc.compile()
    return nc


def _r2(ap):
    """[128, 2*F] view as [128, 2, F] for DoubleRow operands."""
    return ap.rearrange("p (two f) -> p two f", two=2)


def _emit(tc, xs, wm, wv, wp, gnw, gnb, ones_d, gmat_d, out_d,
          loop_reps=None):
    nc = tc.nc
    from contextlib import ExitStack
    ctx = ExitStack()
    with ctx:
        consts = ctx.enter_context(tc.tile_pool(name="consts", bufs=1))
        xin = ctx.enter_context(tc.tile_pool(name="xin", bufs=8))
        scr = ctx.enter_context(tc.tile_pool(name="scr", bufs=4))
        small = ctx.enter_context(tc.tile_pool(name="small", bufs=4))
        hpool = ctx.enter_context(tc.tile_pool(name="hpool", bufs=4))
        tpool = ctx.enter_context(tc.tile_pool(name="tpool", bufs=4))
        vpool = ctx.enter_context(tc.tile_pool(name="vpool", bufs=8))
        apool = ctx.enter_context(tc.tile_pool(name="apool", bufs=8))
        rpool = ctx.enter_context(tc.tile_pool(name="rpool", bufs=2))
        h2pool = ctx.enter_context(tc.tile_pool(name="h2pool", bufs=4))
        psbig = ctx.enter_context(tc.tile_pool(name="psbig", bufs=3, space="PSUM"))
        psgn = ctx.enter_context(tc.tile_pool(name="psgn", bufs=1, space="PSUM"))
        pswarm = ctx.enter_context(tc.tile_pool(name="pswarm", bufs=1, space="PSUM"))

        # ---- constants, loaded once ----
        # consts ride the (otherwise idle) Pool DMA queue so the x loads own SP
        def load_const(tag, src, shape, dt=F32):
            t = consts.tile(shape, dt, tag=tag, name=tag)
            nc.gpsimd.dma_start(out=t, in_=src)
            return t

        gmat_sb = load_const("gmat", gmat_d, [128, 128])   # first: feeds warmup
        wm_sb = [load_const(f"wm{cp}", wm[cp], [128, 2 * C], FP8) for cp in range(CP)]
        wv_sb = [load_const(f"wv{cp}", wv[cp], [128, 2 * C], FP8) for cp in range(CP)]
        wp_sb = [load_const(f"wp{cp}", wp[cp], [128, 2 * C], FP8) for cp in range(CP)]
        gnw_sb = load_const("gnw", gnw, [128, CT])
        gnb_sb = load_const("gnb", gnb, [128, CT])
        ones_sb = load_const("ones", ones_d, [128, 128], BF16)
        ebias_sb = consts.tile([128, 1], F32, tag="ebias", name="ebias")
        nc.vector.memset(ebias_sb, EXP_BIAS)

        def xload(b):
            """x load for batch b, split across the SP and DVE DMA queues
            (the Pool queue carries the ~1MB of consts; sharing it would
            serialize x behind them)."""
            xt = []
            for t in range(CT):
                a = xin.tile([128, N], F32, tag="xt")
                eng = nc.sync if t % 2 == 0 else nc.scalar
                eng.dma_start(out=a, in_=xs[b, t])
                xt.append(a)
            return xt

        def gn_phase(b, xt):
            """GroupNorm for batch b; returns h2t."""
            # ---- GroupNorm stats, split across engines: sum(x^2) via ACT
            # Square+accum (cols 4:8), sum(x) via DVE reduce (cols 0:4); the
            # gmat matmul turns them into [mean | E[x^2]] broadcast across
            # each 16-channel group. ----
            stat = small.tile([128, 24], F32, tag="stat")
            for t in range(CT):
                sq = scr.tile([128, N], FP8, tag="sq")
                nc.scalar.activation(sq, xt[t], Act.Square,
                                     accum_out=stat[:, 4 + t:5 + t])
                nc.vector.reduce_sum(stat[:, t:t + 1], xt[t],
                                     mybir.AxisListType.X)
            gps = psgn.tile([128, 8], F32, tag="gn")
            nc.tensor.matmul(gps, lhsT=gmat_sb, rhs=stat[:, 0:8],
                             start=True, stop=True)
            g = small.tile([128, 20], F32, tag="gst")
            nc.vector.tensor_copy(out=g[:, 0:8], in_=gps)
            mean, ex2 = g[:, 0:4], g[:, 4:8]
            m2, ve, y, c = g[:, 8:12], g[:, 12:16], g[:, 16:20], stat[:, 8:12]
            th, sc, bc = stat[:, 12:16], stat[:, 16:20], stat[:, 20:24]
            nc.vector.tensor_tensor(m2, mean, mean, AluOpType.mult)
            nc.vector.scalar_tensor_tensor(out=ve, in0=ex2, scalar=EPS,
                                           in1=m2, op0=AluOpType.add,
                                           op1=AluOpType.subtract)  # var+eps
            # Newton rsqrt (bit-trick seed + 1 iteration, ~0.2% worst case --
            # far below the fp8 noise floor) on DVE: keeps ACT free of
            # Sqrt/Ln so one table set serves the whole program
            nc.vector.tensor_scalar_mul(th, ve, 0.5)
            nc.vector.tensor_scalar(y.bitcast(I32), ve.bitcast(I32), 1, None,
                                    op0=AluOpType.logical_shift_right)
            nc.vector.tensor_scalar(y.bitcast(I32), y.bitcast(I32),
                                    -1, 0x5f3759df,
                                    op0=AluOpType.mult, op1=AluOpType.add)
            for _ in range(1):
                nc.vector.tensor_tensor(c, y, y, AluOpType.mult)
                nc.vector.tensor_tensor(c, th, c, AluOpType.mult)
                nc.vector.tensor_scalar(c, c, -1.0, 1.5,
                                        op0=AluOpType.mult, op1=AluOpType.add)
                nc.vector.tensor_tensor(y, y, c, AluOpType.mult)
            nc.vector.tensor_tensor(sc, y, gnw_sb, AluOpType.mult)      # s
            nc.vector.tensor_tensor(bc, mean, sc, AluOpType.mult)       # mean*s
            nc.vector.tensor_tensor(bc, gnb_sb, bc, AluOpType.subtract)  # b'

            # ---- h = s*x + b', straight to fp8 DoubleRow layout ----
            h2t = [hpool.tile([128, 2 * N], FP8, tag="h", name="h2t")
                   for _ in range(CP)]
            for t in range(CT):
                nc.vector.tensor_scalar(h2t[t // 2][:, (t % 2) * N:(t % 2 + 1) * N],
                                        xt[t], sc[:, t:t + 1], bc[:, t:t + 1],
                                        op0=AluOpType.mult, op1=AluOpType.add)
            return h2t

        def t_phase(h2t):
            """t = M h (the key-side vectors): k2-style layout, one tile
            pair per channel-pair, full 1024 key positions; psum halves =
            (nch0 | nch1)."""
            t2 = [tpool.tile([128, 2 * N], FP8, tag="t", name="t2")
                  for _ in range(CP)]
            for dt in range(CT):
                dsl = slice(128 * dt, 128 * (dt + 1))
                ps = psbig.tile([128, 1024], F32, tag="ps")
                for cp in range(CP):
                    lw = _r2(wm_sb[cp])[:, :, dsl]
                    for nch in range(NCH):
                        nsl = slice(512 * nch, 512 * (nch + 1))
                        nc.tensor.matmul(ps[:, 512 * nch:512 * (nch + 1)],
                                         lhsT=lw,
                                         rhs=_r2(h2t[cp])[:, :, nsl],
                                         start=(cp == 0), stop=(cp == CP - 1),
                                         perf_mode=DR)
                nc.scalar.activation(
                    t2[dt // 2][:, (dt % 2) * N:(dt % 2 + 1) * N], ps,
                    Act.Identity)
            return t2

        def v_group(h2t, np_):
            """One v^T tile pair (key tiles 2np_, 2np_+1), bf16 out."""
            ps = psbig.tile([128, 1024], F32, tag="ps")
            for i2 in range(2):
                psl = slice(128 * (2 * np_ + i2), 128 * (2 * np_ + i2 + 1))
                for cp in range(CP):
                    nc.tensor.matmul(ps[:, 512 * i2:512 * (i2 + 1)],
                                     lhsT=_r2(h2t[cp])[:, :, psl],
                                     rhs=_r2(wv_sb[cp]),
                                     start=(cp == 0), stop=(cp == CP - 1),
                                     perf_mode=DR)
            vt = vpool.tile([128, 1024], BF16, tag="vt")
            nc.vector.tensor_copy(out=vt, in_=ps)
            return vt

        def scores_span(h2t, t2, at2, vt2, pts):
            """scores^T[key, query] = t^T h + exp -> bf16 for the given key
            tiles, with the V projection's matmul groups interleaved after
            odd key tiles: the scores phase is exp-rate-limited on ACT, and
            the independent V matmuls soak up the PE bubbles (strict-FIFO
            engine queues make emission order the execution order)."""
            for pt in pts:
                ksl = slice(128 * pt, 128 * (pt + 1))
                ps = psbig.tile([128, 1024], F32, tag="ps")
                for cp in range(CP):
                    lt = _r2(t2[cp])[:, :, ksl]
                    for nch in range(NCH):
                        nsl = slice(512 * nch, 512 * (nch + 1))
                        nc.tensor.matmul(ps[:, 512 * nch:512 * (nch + 1)],
                                         lhsT=lt, rhs=_r2(h2t[cp])[:, :, nsl],
                                         start=(cp == 0), stop=(cp == CP - 1),
                                         perf_mode=DR)
                nc.scalar.activation(
                    at2[pt // 2][:, (pt % 2) * N:(pt % 2 + 1) * N], ps,
                    Act.Exp, scale=ATTN_SCALE / SM, bias=ebias_sb)
                if pt % 2 == 1:
                    vt2.append(v_group(h2t, (pt - 1) // 2))

        def denom_phase(at2):
            """softmax denominator: all-ones bf16 matmul -> key-sum broadcast
            on all partitions; one reciprocal for both chunks."""
            psd = psbig.tile([128, 1024], F32, tag="ps")
            for nch in range(NCH):
                for nt in range(NT):
                    asl = slice((nt % 2) * N + 512 * nch,
                                (nt % 2) * N + 512 * (nch + 1))
                    nc.tensor.matmul(psd[:, 512 * nch:512 * (nch + 1)],
                                     lhsT=ones_sb,
                                     rhs=at2[nt // 2][:, asl],
                                     start=(nt == 0), stop=(nt == NT - 1))
            rc = rpool.tile([128, 1024], F32, tag="rc")
            nc.vector.reciprocal(out=rc, in_=psd)
            return rc

        def av_group(at2, vt2, rc, h2q, ct):
            """AV (bf16, normal mode) for one output channel tile; psum
            halves = (nch0 | nch1); normalize by 1/denom on evacuation."""
            ps = psbig.tile([128, 1024], F32, tag="ps")
            for nt in range(NT):
                lv = vt2[nt // 2][:, (nt % 2) * 512 + 128 * ct:
                                  (nt % 2) * 512 + 128 * (ct + 1)]
                for nch in range(NCH):
                    asl = slice((nt % 2) * N + 512 * nch,
                                (nt % 2) * N + 512 * (nch + 1))
                    nc.tensor.matmul(ps[:, 512 * nch:512 * (nch + 1)],
                                     lhsT=lv, rhs=at2[nt // 2][:, asl],
                                     start=(nt == 0), stop=(nt == NT - 1))
            nc.vector.tensor_tensor(
                h2q[ct // 2][:, (ct % 2) * N:(ct % 2 + 1) * N],
                ps, rc, AluOpType.mult)

        def proj_group(b, xt, h2q, dt):
            """proj (fp8 DR) + residual (in place into xt) + store for one
            output channel tile; 512-wide evacuation halves."""
            dsl = slice(128 * dt, 128 * (dt + 1))
            ps = psbig.tile([128, 1024], F32, tag="ps")
            for cp in range(CP):
                lw = _r2(wp_sb[cp])[:, :, dsl]
                for nch in range(NCH):
                    qsl = slice(512 * nch, 512 * (nch + 1))
                    nc.tensor.matmul(ps[:, 512 * nch:512 * (nch + 1)],
                                     lhsT=lw, rhs=_r2(h2q[cp])[:, :, qsl],
                                     start=(cp == 0), stop=(cp == CP - 1),
                                     perf_mode=DR)
            for nch in range(NCH):
                qsl = slice(512 * nch, 512 * (nch + 1))
                nc.vector.scalar_tensor_tensor(
                    out=xt[dt][:, qsl], in0=ps[:, qsl],
                    scalar=1.0 / (SW * SW), in1=xt[dt][:, qsl],
                    op0=AluOpType.mult, op1=AluOpType.add)
            nc.gpsimd.dma_start(out=out_d[b, dt], in_=xt[dt])

        def warmup():
            """~7us of throwaway fp32 matmuls over a memset tile (no DMA
            dependency): runs during the first GroupNorm (PE otherwise idle)
            so the HAM clock gate reaches 8/8 -- and stays there -- before
            the real matmuls arrive."""
            wsrc = consts.tile([128, 512], F32, tag="wsrc", name="wsrc")
            nc.vector.memset(wsrc, 0.125)
            for i in range(8):
                ps = pswarm.tile([128, 512], F32, tag="warm", name="warm")
                nc.tensor.matmul(ps, lhsT=wsrc[:, 0:128], rhs=wsrc,
                                 start=True, stop=True)

        def new_at2():
            return [apool.tile([128, 2 * N], BF16, tag="at", name="at2")
                    for _ in range(KP)]

        def new_h2q():
            return [h2pool.tile([128, 2 * N], FP8, tag="h2", name="h2q")
                    for _ in range(CP)]

        def body():
            # Deep cross-batch software pipeline. Engine queues are strict
            # FIFO, so emission order IS execution order: batch 1's
            # GN/T/scores/V work is woven into batch 0's denominator and AV
            # phases to fill every PE wait.
            # HW A/B: warmup measured 128.8us vs 107.7us without -- the
            # WAW-serialized warm matmuls block the PE FIFO far worse on
            # hardware than the cost model predicts. Keep it off.
            if os.environ.get("KERNEL_WARMUP", "0") == "1":
                warmup()
            x0 = xload(0)
            h0 = gn_phase(0, x0)
            t20 = t_phase(h0)
            at0, vt0 = new_at2(), []
            scores_span(h0, t20, at0, vt0, range(NT))
            x1 = xload(1)
            h1 = gn_phase(1, x1)
            rc0 = denom_phase(at0)
            t21 = t_phase(h1)                       # fills denom/recip wait
            h2q0 = new_h2q()
            av_group(at0, vt0, rc0, h2q0, 0)
            av_group(at0, vt0, rc0, h2q0, 1)
            at1, vt1 = new_at2(), []
            scores_span(h1, t21, at1, vt1, range(0, 4))
            av_group(at0, vt0, rc0, h2q0, 2)
            av_group(at0, vt0, rc0, h2q0, 3)
            scores_span(h1, t21, at1, vt1, range(4, NT))
            for dt in range(CT):
                proj_group(0, x0, h2q0, dt)
            rc1 = denom_phase(at1)
            h2q1 = new_h2q()
            for ct in range(CT):
                av_group(at1, vt1, rc1, h2q1, ct)
            for dt in range(CT):
                proj_group(1, x1, h2q1, dt)

        if loop_reps is None:
            body()
        else:
            with tc.For_i(0, loop_reps, 1):
                body()


def _pack_w(w, scale):
    """W[d_out, c_in] -> DoubleRow stationary layout [CP, 128, 2*C] fp8,
    scaled. [cp][p, ko*C + d] = W.T[cp*256 + ko*128 + p, d] * scale."""
    f8 = mybir.dt.np(FP8)
    wT = np.ascontiguousarray(np.asarray(w, np.float32).T) * scale
    wT = np.clip(wT, -224.0, 224.0)
    return np.ascontiguousarray(
        wT.reshape(CP, 2, 128, C).transpose(0, 2, 1, 3).reshape(CP, 128, 2 * C)
    ).astype(f8)


def _prep_inputs(x, gn_w, gn_b, q_w, q_b, k_w, k_b, v_w, v_b, p_w, p_b):
    f = np.float32
    for name, bias in (("q_b", q_b), ("k_b", k_b), ("v_b", v_b), ("p_b", p_b)):
        if np.any(np.asarray(bias)):
            raise NotImplementedError(f"nonzero {name} not supported")
    x = np.ascontiguousarray(np.asarray(x, f)).reshape(B, CT, 128, N)
    m = np.asarray(q_w, f).T @ np.asarray(k_w, f)   # scores = h^T (M h)
    base = {
        "wm2": _pack_w(m, SM),
        "wv2": _pack_w(v_w, SW),
        "wp2": _pack_w(p_w, SW),
        "gnw": np.ascontiguousarray(np.asarray(gn_w, f).reshape(CT, 128).T),
        "gnb": np.ascontiguousarray(np.asarray(gn_b, f).reshape(CT, 128).T),
        "ones1": np.ones((128, 128), f).astype(mybir.dt.np(BF16)),
        # block-diagonal group-averaging matrix, scaled so the matmul yields
        # means directly: G[p, m] = 1/(GS*N) iff p//GS == m//GS
        "gmat": np.ascontiguousarray(
            np.kron(np.eye(128 // GS, dtype=f), np.ones((GS, GS), f)) / (GS * N)),
    }
    return x, base, ()


def kernel(x, temb, gn_w, gn_b, q_w, q_b, k_w, k_b, v_w, v_b, p_w, p_b):
    global LAST_RESULTS
    del temb  # unused by the reference module
    x_r, base, flags = _prep_inputs(x, gn_w, gn_b, q_w, q_b, k_w, k_b,
                                    v_w, v_b, p_w, p_b)
    if flags not in _PROGRAM_CACHE:
        _PROGRAM_CACHE[flags] = _build_program(flags)
    nc = _PROGRAM_CACHE[flags]

    in_maps = [dict(base, xs=np.ascontiguousarray(x_r[BPC * i: BPC * (i + 1)]))
               for i in range(NCORES)]
    res = run_bass_kernel_spmd(nc, in_maps, core_ids=list(range(NCORES)))
    LAST_RESULTS = res
    out = np.concatenate([r["out"] for r in res.results], axis=0)
    return np.ascontiguousarray(out.reshape(B, C, H, W).astype(np.float32))
